# revision 22
# baseline (speedup 1.0000x reference)
"""Trainium2 Bass kernel for BoxConv2d (nn_BoxConv2d_47880295416171).

BoxConv2d is linear and separable in the input image: folding the
integral-image cumsum into the bilinear corner-interpolation gives, per
(channel c, filter f),

    out[b,c,f] = U[c,f] @ input[b,c] @ V[c,f]^T

where U[c,f,x,i] = clip(xs_b(x)-i,0,1) - clip(xs_t(x)-i,0,1) (a soft box-row
indicator, area-normalized) and V likewise along W.  U/V depend only on the
tiny [32,4] box parameters and are built on the host; the heavy work (two
128x128x128 matmuls per output plane, 2048 matmuls total) runs on the
TensorEngines.

Sharding: output-channel parallel over c (32 channels / 8 cores = 4 per
core).  Each core processes all 8 batches for its 4 channels, so per-core
weight traffic is only its own U/V slice (vs 8x if batch-sharded).
"""
import contextlib

import numpy as np

import concourse.bass as bass
import concourse.tile as tile
from concourse import bacc, masks, mybir
from concourse.bass_utils import run_bass_kernel_spmd

B, C, F, H, W = 8, 32, 4, 128, 128
RH = RW = 1024.0
N_CORES = 8
CP = C // N_CORES  # channels per core = 4
FW = F * W         # 512


def _build_uv(x_min, x_max, y_min, y_max):
    xmin = np.asarray(x_min, np.float64) * RH
    xmax = np.asarray(x_max, np.float64) * RH
    ymin = np.asarray(y_min, np.float64) * RW
    ymax = np.asarray(y_max, np.float64) * RW

    hx = np.arange(H, dtype=np.float64)
    wy = np.arange(W, dtype=np.float64)
    xs_t = np.clip(hx[None, None, :] + xmin[:, :, None], 0.0, H)
    xs_b = np.clip(hx[None, None, :] + xmax[:, :, None] + 1.0, 0.0, H)
    ys_l = np.clip(wy[None, None, :] + ymin[:, :, None], 0.0, W)
    ys_r = np.clip(wy[None, None, :] + ymax[:, :, None] + 1.0, 0.0, W)

    i = np.arange(H, dtype=np.float64)
    j = np.arange(W, dtype=np.float64)
    U = (np.clip(xs_b[..., None] - i, 0.0, 1.0)
         - np.clip(xs_t[..., None] - i, 0.0, 1.0))      # [C,F,H(x),H(i)]
    V = (np.clip(ys_r[..., None] - j, 0.0, 1.0)
         - np.clip(ys_l[..., None] - j, 0.0, 1.0))      # [C,F,W(y),W(j)]
    area = (xmax - xmin + 1.0) * (ymax - ymin + 1.0)
    U = U / area[:, :, None, None]
    return U.astype(np.float32), V.astype(np.float32)


MM_DT = "f32r"  # matmul precision: "f32" | "bf16" | "f32r"


def _build_bass_f32r(loop_n=1):
    """fp32r variant: full-rate PE at (near-)fp32 precision.  mm1 has moving
    dim 512; mm2 pairs two batches side-by-side for moving dim 256 (fp32r
    runs 1 cyc/row only at >=256).  loop_n>1 repeats the whole body on-device
    (benchmark mode)."""
    f32 = mybir.dt.float32
    fr = mybir.dt.float32r
    nc = bacc.Bacc("TRN2", target_bir_lowering=False, debug=False,
                   enable_asserts=False, num_devices=N_CORES)
    inp_d = nc.dram_tensor("inp", [B, CP, H, W], fr, kind="ExternalInput")
    ut_d = nc.dram_tensor("ut", [H, CP * F * H], fr, kind="ExternalInput")
    vt_d = nc.dram_tensor("vt", [W, CP * FW], fr, kind="ExternalInput")
    ident_d = nc.dram_tensor("ident", [128, 128], fr, kind="ExternalInput")
    out_d = nc.dram_tensor("out", [B, CP * F, H, W], f32, kind="ExternalOutput")

    with tile.TileContext(nc) as tc:
        with (
            tc.tile_pool(name="const", bufs=1) as cpool,
            tc.tile_pool(name="wts", bufs=1) as wpool,
            tc.tile_pool(name="inio", bufs=8) as iopool,
            tc.tile_pool(name="intp", bufs=4) as itpool,
            tc.tile_pool(name="work", bufs=3) as work,
            tc.tile_pool(name="outio", bufs=32) as opool,
            tc.tile_pool(name="pstp", bufs=3, space="PSUM") as pstp,
            tc.tile_pool(name="psa", bufs=3, space="PSUM") as psa,
            tc.tile_pool(name="pso", bufs=2, space="PSUM") as pso,
        ):
            ident = cpool.tile([128, 128], fr)
            nc.sync.dma_start(ident[:], ident_d[:])  # f32r memset is invalid ISA

            ut_t = wpool.tile([128, CP * F * H], fr)
            nc.sync.dma_start(ut_t[:], ut_d[:])
            vt_t = wpool.tile([128, CP * FW], fr)
            nc.sync.dma_start(vt_t[:], vt_d[:])

            rep = (tc.For_i(0, loop_n, 1, name="rep") if loop_n > 1
                   else contextlib.nullcontext())
            with rep:
                _f32r_body(nc, tc, inp_d, out_d, ut_t, vt_t, ident,
                           iopool, itpool, work, opool, pstp, psa, pso)
    nc.compile()
    return nc


def _f32r_body(nc, tc, inp_d, out_d, ut_t, vt_t, ident,
               iopool, itpool, work, opool, pstp, psa, pso):
    f32 = mybir.dt.float32
    fr = mybir.dt.float32r
    if True:
            for bp in range(B // 2):
                inpTs = []
                for s in range(2):
                    b = 2 * bp + s
                    in_b = iopool.tile([128, CP * W], fr)
                    nc.gpsimd.dma_start(
                        in_b[:].rearrange("i (c j) -> i c j", c=CP),
                        inp_d[b].rearrange("c i j -> i c j"),
                    )
                    tpB = pstp.tile([128, CP * W], fr)
                    for cp in range(CP):
                        nc.tensor.transpose(
                            tpB[:, cp * W:(cp + 1) * W],
                            in_b[:, cp * W:(cp + 1) * W], ident[:],
                        )
                    inpT = itpool.tile([128, CP * W], fr)
                    if s == 0:
                        nc.vector.tensor_copy(inpT[:], tpB[:])
                    else:
                        nc.scalar.copy(inpT[:], tpB[:])
                    inpTs.append(inpT)

                for cp in range(CP):
                    it = bp * CP + cp
                    # A_s[i,(f,y)]; evacuate into paired layout [i,(s,f,y)]
                    a2 = work.tile([128, 2, F, W], fr, tag="a2")
                    for s in range(2):
                        ps_a = psa.tile([128, FW], f32)
                        nc.tensor.matmul(
                            ps_a[:], inpTs[s][:, cp * W:(cp + 1) * W],
                            vt_t[:, cp * FW:(cp + 1) * FW],
                            start=True, stop=True,
                        )
                        dst = a2[:, s]  # [p, f, y] contiguous block
                        src = ps_a[:].rearrange("p (f y) -> p f y", f=F)
                        if (it + s) % 2 == 0:
                            nc.vector.tensor_copy(dst, src)
                        else:
                            nc.scalar.copy(dst, src)

                    # out[x,(f2,s,y)]: moving dim 256 per matmul (s,y pair)
                    for g in range(2):          # f-groups {0,1}, {2,3}
                        ps_o = pso.tile([128, 2, 2, W], f32)
                        for h in range(2):
                            f = 2 * g + h
                            k = (cp * F + f) * H
                            nc.tensor.matmul(
                                ps_o[:, h], ut_t[:, k:k + H], a2[:, :, f, :],
                                start=True, stop=True,
                            )
                        o_sb = opool.tile([128, 2, 2, W], f32)
                        if (it + g) % 2 == 0:
                            nc.vector.tensor_copy(o_sb[:], ps_o[:])
                        else:
                            nc.scalar.copy(o_sb[:], ps_o[:])
                        # DMA APs are limited to 3 dims: one store per s
                        for s in range(2):
                            eng = nc.sync if (it + g + s) % 2 == 0 else nc.gpsimd
                            eng.dma_start(
                                out_d[2 * bp + s,
                                      cp * F + 2 * g:cp * F + 2 * g + 2]
                                .rearrange("f x y -> x f y"),
                                o_sb[:, :, s, :],
                            )


def _build_bass(mm_dt=None):
    mm_dt = mm_dt or MM_DT
    if mm_dt == "f32r":
        return _build_bass_f32r()
    f32 = mybir.dt.float32
    mdt = {"f32": mybir.dt.float32, "bf16": mybir.dt.bfloat16}[mm_dt]
    nc = bacc.Bacc("TRN2", target_bir_lowering=False, debug=False,
                   enable_asserts=False, num_devices=N_CORES)
    inp_d = nc.dram_tensor("inp", [B, CP, H, W], f32, kind="ExternalInput")
    ut_d = nc.dram_tensor("ut", [H, CP * F * H], mdt, kind="ExternalInput")
    vt_d = nc.dram_tensor("vt", [W, CP * FW], mdt, kind="ExternalInput")
    out_d = nc.dram_tensor("out", [B, CP * F, H, W], f32, kind="ExternalOutput")

    with tile.TileContext(nc) as tc:
        with (
            tc.tile_pool(name="const", bufs=1) as cpool,
            tc.tile_pool(name="wts", bufs=1) as wpool,
            tc.tile_pool(name="inio", bufs=8) as iopool,
            tc.tile_pool(name="intp", bufs=3) as itpool,
            tc.tile_pool(name="work", bufs=6) as work,
            tc.tile_pool(name="outio", bufs=32) as opool,
            tc.tile_pool(name="pstp", bufs=2, space="PSUM") as pstp,
            tc.tile_pool(name="psa", bufs=3, space="PSUM") as psa,
            tc.tile_pool(name="pso", bufs=3, space="PSUM") as pso,
        ):
            ident = cpool.tile([128, 128], mdt)
            masks.make_identity(nc, ident[:])

            ut_t = wpool.tile([128, CP * F * H], mdt)
            nc.sync.dma_start(ut_t[:], ut_d[:])
            vt_t = wpool.tile([128, CP * FW], mdt)
            nc.sync.dma_start(vt_t[:], vt_d[:])

            for b in range(B):
                in_b = iopool.tile([128, CP * W], mdt)
                if mm_dt == "f32":
                    nc.sync.dma_start(
                        in_b[:].rearrange("i (c j) -> i c j", c=CP),
                        inp_d[b].rearrange("c i j -> i c j"),
                    )
                else:
                    # SWDGE casts fp32->bf16 during the DMA itself
                    nc.gpsimd.dma_start(
                        in_b[:].rearrange("i (c j) -> i c j", c=CP),
                        inp_d[b].rearrange("c i j -> i c j"),
                    )
                # transpose all 4 channel planes into one PSUM tile, then
                # evacuate with a single wide (2x-mode bf16) copy
                tpB = pstp.tile([128, CP * W], mdt)
                for cp in range(CP):
                    nc.tensor.transpose(
                        tpB[:, cp * W:(cp + 1) * W],
                        in_b[:, cp * W:(cp + 1) * W], ident[:],
                    )
                inpT = itpool.tile([128, CP * W], mdt)
                nc.vector.tensor_copy(inpT[:], tpB[:])

                for cp in range(CP):
                    it = b * CP + cp
                    # A[i,(f,y)] = sum_j inpT[j,i] * VT[j,(f,y)]
                    ps_a = psa.tile([128, FW], f32)
                    nc.tensor.matmul(
                        ps_a[:], inpT[:, cp * W:(cp + 1) * W],
                        vt_t[:, cp * FW:(cp + 1) * FW],
                        start=True, stop=True,
                    )
                    a_sb = work.tile([128, FW], mdt, tag="a")
                    if it % 8 == 7:
                        nc.scalar.copy(a_sb[:], ps_a[:])
                    else:
                        nc.vector.tensor_copy(a_sb[:], ps_a[:])

                    # out[x,(f,y)] = sum_i UT[i,x] * A[i,(f,y)] per f
                    ps_o = pso.tile([128, FW], f32)
                    for f in range(F):
                        k = (cp * F + f) * H
                        nc.tensor.matmul(
                            ps_o[:, f * W:(f + 1) * W],
                            ut_t[:, k:k + H],
                            a_sb[:, f * W:(f + 1) * W],
                            start=True, stop=True,
                        )
                    o_sb = opool.tile([128, FW], f32)
                    nc.scalar.copy(o_sb[:], ps_o[:])
                    # spread output DMA across the SP and Pool rings (ACT is
                    # busy with PSUM evacuation copies)
                    eng = nc.sync if it % 2 == 0 else nc.gpsimd
                    eng.dma_start(
                        out_d[b, cp * F:(cp + 1) * F].rearrange("f x y -> x f y"),
                        o_sb[:].rearrange("x (f y) -> x f y", f=F),
                    )
    nc.compile()  # bacc legalization: splits waits to the 1-per-inst HW limit
    return nc


def _in_maps(inputs, mm_dt=None):
    mm_dt = mm_dt or MM_DT
    inp = np.ascontiguousarray(np.asarray(inputs["input"], np.float32))
    U, V = _build_uv(inputs["x_min"], inputs["x_max"],
                     inputs["y_min"], inputs["y_max"])
    if mm_dt == "bf16":
        import ml_dtypes
        wdt = ml_dtypes.bfloat16
    else:
        wdt = np.float32
    maps = []
    for k in range(N_CORES):
        cs = slice(CP * k, CP * (k + 1))
        # ut[i, (c',f,x)] = U[c,f,x,i];  vt[j, (c',f,y)] = V[c,f,y,j]
        ut = np.ascontiguousarray(
            U[cs].transpose(3, 0, 1, 2).reshape(H, CP * F * H)).astype(wdt)
        vt = np.ascontiguousarray(
            V[cs].transpose(3, 0, 1, 2).reshape(W, CP * FW)).astype(wdt)
        m = {
            "inp": np.ascontiguousarray(inp[:, cs]),
            "ut": ut,
            "vt": vt,
        }
        if mm_dt == "f32r":
            m["ident"] = np.eye(128, dtype=np.float32)
        maps.append(m)
    return maps


def run(inputs, trace=False, **kw):
    """Shard, run on 8 cores, gather. Returns (output, BassKernelResults)."""
    nc = _build_bass()
    res = run_bass_kernel_spmd(nc, _in_maps(inputs),
                               core_ids=list(range(N_CORES)),
                               trace=trace, **kw)
    out = np.concatenate([res.results[k]["out"] for k in range(N_CORES)],
                         axis=1)
    return out, res


def _null_bass():
    """Minimal 8-core program: one 64KB DMA through SBUF per core."""
    f32 = mybir.dt.float32
    nc = bacc.Bacc("TRN2", target_bir_lowering=False, debug=False,
                   enable_asserts=False, num_devices=N_CORES)
    x = nc.dram_tensor("x", [128, 128], f32, kind="ExternalInput")
    y = nc.dram_tensor("y", [128, 128], f32, kind="ExternalOutput")
    with tile.TileContext(nc) as tc:
        with tc.tile_pool(name="p", bufs=1) as p:
            t = p.tile([128, 128], f32)
            nc.sync.dma_start(t[:], x[:])
            nc.sync.dma_start(y[:], t[:])
    nc.compile()
    return nc


def _make_timed(nc, in_maps):
    """Replicate bass2jax.run_bass_via_pjrt's lowering without donation;
    return (fn, device_args) for repeated timed execution."""
    import jax
    from jax.sharding import Mesh, NamedSharding, PartitionSpec
    from jax.experimental.shard_map import shard_map
    from concourse import bass2jax, mybir as mb

    bass2jax.install_neuronx_cc_hook()
    partition_name = (nc.partition_id_tensor.name
                      if nc.partition_id_tensor else None)
    in_names, out_names, out_avals, zero_outs = [], [], [], []
    for alloc in nc.m.functions[0].allocations:
        if not isinstance(alloc, mb.MemoryLocationSet):
            continue
        name = alloc.memorylocations[0].name
        if alloc.kind == "ExternalInput":
            if name != partition_name:
                in_names.append(name)
        elif alloc.kind == "ExternalOutput":
            out_names.append(name)
            shape = tuple(alloc.tensor_shape)
            dtype = mb.dt.np(alloc.dtype)
            out_avals.append(jax.core.ShapedArray(shape, dtype))
            zero_outs.append(np.zeros(shape, dtype))
    n_params = len(in_names)
    all_names = in_names + out_names
    if partition_name is not None:
        all_names = all_names + [partition_name]

    def _body(*args):
        operands = list(args)
        if partition_name is not None:
            operands.append(bass2jax.partition_id_tensor())
        outs = bass2jax._bass_exec_p.bind(
            *operands,
            out_avals=tuple(out_avals),
            in_names=tuple(all_names),
            out_names=tuple(out_names),
            lowering_input_output_aliases=(),
            sim_require_finite=True,
            sim_require_nnan=True,
            nc=nc,
        )
        return tuple(outs)

    devices = jax.devices()[:N_CORES]
    mesh = Mesh(np.asarray(devices), ("core",))
    spec = PartitionSpec("core")
    n_all = n_params + len(out_names)
    fn = jax.jit(
        shard_map(_body, mesh=mesh, in_specs=(spec,) * n_all,
                  out_specs=(spec,) * len(out_names), check_rep=False),
        keep_unused=True,
    )
    concat_in = [
        np.concatenate([np.asarray(m[name]) for m in in_maps], axis=0)
        for name in in_names
    ]
    concat_zeros = [
        np.zeros((N_CORES * z.shape[0], *z.shape[1:]), z.dtype)
        for z in zero_outs
    ]
    sh = NamedSharding(mesh, spec)
    dev_args = [jax.device_put(a, sh) for a in concat_in + concat_zeros]
    return fn, dev_args


def bench(inputs, iters=50):
    """Time the kernel with device-resident args; subtract a null-kernel
    baseline to remove axon dispatch overhead. Returns dict of stats."""
    import time
    import jax

    stats = {}
    for tag, nc, maps in (
        ("null", _null_bass(),
         [{"x": np.zeros((128, 128), np.float32)} for _ in range(N_CORES)]),
        ("kernel", _build_bass(), _in_maps(inputs)),
    ):
        fn, args = _make_timed(nc, maps)
        jax.block_until_ready(fn(*args))  # compile + warm
        jax.block_until_ready(fn(*args))
        times = []
        for _ in range(iters):
            t0 = time.perf_counter()
            jax.block_until_ready(fn(*args))
            times.append(time.perf_counter() - t0)
        times = np.array(times)
        stats[tag] = {"mean": times.mean(), "min": times.min(),
                      "p50": float(np.median(times))}
    for k in ("mean", "min", "p50"):
        stats[f"delta_{k}_ns"] = (stats["kernel"][k] - stats["null"][k]) * 1e9
    return stats


def kernel(input, x_min, x_max, y_min, y_max):
    out, _ = run({"input": input, "x_min": x_min, "x_max": x_max,
                  "y_min": y_min, "y_max": y_max})
    return out


def bench_loop(inputs, n1=256, n2=1024, iters=30):
    """HW timing via on-device repetition: two compiles of the same program
    with loop_n=n1 and loop_n=n2; per-iteration time = delta/(n2-n1).
    Dispatch/transfer overhead cancels exactly."""
    import time
    import jax

    maps = _in_maps(inputs)
    res = {}
    for n in (n1, n2):
        nc = _build_bass_f32r(loop_n=n) if MM_DT == "f32r" else None
        assert nc is not None, "bench_loop only wired for f32r"
        fn, args = _make_timed(nc, maps)
        jax.block_until_ready(fn(*args))
        jax.block_until_ready(fn(*args))
        ts = []
        for _ in range(iters):
            t0 = time.perf_counter()
            jax.block_until_ready(fn(*args))
            ts.append(time.perf_counter() - t0)
        ts = np.array(ts)
        res[n] = {"p50": float(np.median(ts)), "mean": ts.mean(),
                  "min": ts.min()}
        print(f"  loop_n={n}: p50 {res[n]['p50']*1e3:.1f}ms "
              f"min {res[n]['min']*1e3:.1f}ms mean {res[n]['mean']*1e3:.1f}ms")
    dn = n2 - n1
    return {k: (res[n2][k] - res[n1][k]) / dn * 1e9 for k in ("p50", "mean", "min")}
